# revision 18
# baseline (speedup 1.0000x reference)
"""Grouped-query attention (B=2,T=2048,D=2048, 4 groups x 4 heads x 128d) on 8 trn2 cores.

Sharding: core = (batch b, group g); b = core//4, g = core%4 (data parallel x tensor
parallel). Each core computes its group's QKV projections, QK-rmsnorm+rope, causal
flash-style attention, and a partial output projection o_g @ wo_g; the host sums the
4 per-group partials per batch (row-sharded wo all-reduce done on host at gather).

Device layout notes (per core):
  xt   [D, T] bf16 = x[b].T  -> projections produce qT/kT/vT [n, T] with head-dim on
       partitions, which feeds QK^T directly (scores transposed: [j, i], softmax sums
       over partitions via ones-matmul, PV uses v natural as lhsT).
  All matmuls bf16 with f32 PSUM accumulation. Softmax without max-subtraction:
  |scores| <= sqrt(128) by Cauchy-Schwarz after rmsnorm, so exp is safe in f32.

Perf structure (v3):
  - K and V projections run kc-outer interleaved so the PE stays fed while the
    8.4MB xt streams in (a single projection consumes chunks faster than DMA).
  - rmsnorm square/gain read the projection PSUMs directly on the scalar engine
    (no PSUM->SBUF raw copies); rsqrt and softmax denominators are computed via
    ones[128,128]-lhsT matmul chains so the PSUM result is already broadcast
    across partitions (no DRAM-roundtrip broadcasts anywhere).
  - V is transposed to natural layout with DMA xbar transposes (no PE/PSUM use).
  - output projection of i-chunk N is interleaved as filler matmuls into the
    QK/exp/PV stream of i-chunk N+1 (hides exp latency stalls); the last chunk's
    chains alternate between two PSUM rings to pipeline the tail.
  - output written bf16; host sums the 4 group partials per batch in f32.
"""

import sys
from contextlib import ExitStack

for _p in ("/opt/trn_rl_repo", "/opt/pypackages"):
    if _p not in sys.path:
        sys.path.insert(0, _p)

import numpy as np
import ml_dtypes

import concourse.bass as bass
import concourse.mybir as mybir
import concourse.tile as tile
from concourse import bacc
from concourse.bass_utils import run_bass_kernel_spmd

bf16 = ml_dtypes.bfloat16
BF = mybir.dt.bfloat16
F32 = mybir.dt.float32
AF = mybir.ActivationFunctionType

B, T, D = 2, 2048, 2048
HD, H, G = 128, 4, 4
KC = D // 128          # 16 contraction chunks
TB = T // 128          # 16 t blocks
IC = T // 512          # 4 i chunks
EPS = 1e-6
MULT2 = float(HD) ** -0.5   # mult^2 folded into q gains

_NC_CACHE = {}


def _build_nc():
    nc = bacc.Bacc(None)

    xt_d = nc.declare_dram_parameter("xt", [D, T], BF, isOutput=False)
    wq_d = nc.declare_dram_parameter("wq", [D, H * HD], BF, isOutput=False)
    wk_d = nc.declare_dram_parameter("wk", [D, HD], BF, isOutput=False)
    wv_d = nc.declare_dram_parameter("wv", [D, HD], BF, isOutput=False)
    wo_d = nc.declare_dram_parameter("wo", [H * HD, D], BF, isOutput=False)
    gqs_d = nc.declare_dram_parameter("gqs", [HD, H], F32, isOutput=False)
    gks_d = nc.declare_dram_parameter("gks", [HD, 1], F32, isOutput=False)
    cos_d = nc.declare_dram_parameter("cosf", [HD, T], BF, isOutput=False)
    sin_d = nc.declare_dram_parameter("sins", [HD, T], BF, isOutput=False)
    msk_d = nc.declare_dram_parameter("mask", [128, 128], F32, isOutput=False)
    out_d = nc.declare_dram_parameter("out", [T, D], BF, isOutput=True)

    with tile.TileContext(nc) as tc:
        with ExitStack() as outer:
            persist = outer.enter_context(tc.tile_pool(name="persist", bufs=1))
            qhat = [persist.tile([128, T], BF, tag=f"qhat{h}", name=f"qhat{h}") for h in range(H)]
            khat = persist.tile([128, T], BF, tag="khat", name="khat")
            vnat = persist.tile([128, T], BF, tag="vnat", name="vnat")  # [j-local, tb*128+d]
            gqs = persist.tile([HD, H], F32, tag="gqs", name="gqs")
            gks = persist.tile([HD, 1], F32, tag="gks", name="gks")
            ones128 = persist.tile([128, 128], BF, tag="ones", name="ones128")
            eps128 = persist.tile([128, 1], F32, tag="eps", name="eps128")
            wo_t = [persist.tile([128, D], BF, tag=f"wo{h}", name=f"wo{h}") for h in range(H)]
            mask = persist.tile([128, 128], F32, tag="mask", name="mask")
            sq3 = persist.tile([128, T], BF, tag="sq3", name="sq3")
            t13 = persist.tile([128, T], BF, tag="t13", name="t13")

            nc.vector.memset(ones128, 1.0)
            nc.vector.memset(eps128, EPS)

            # ---------------- Phase 1: projections + rmsnorm + rope ----------------
            with ExitStack() as s1:
                xt_p = s1.enter_context(tc.tile_pool(name="xt", bufs=1))
                w_p = s1.enter_context(tc.tile_pool(name="w", bufs=1))
                tmp_p = s1.enter_context(tc.tile_pool(name="tmp", bufs=1))
                ps_proj = s1.enter_context(tc.tile_pool(name="ps_proj", bufs=6, space="PSUM"))

                # wk/wv/xt interleaved so the K+V projection chain starts after the
                # first chunk; cos/sin mid-stream (needed ~20us in); wq next
                # (needed ~30us in); wo/mask last (needed only at attention).
                cosf = w_p.tile([HD, T], BF, tag="cosf", name="cosf")
                sins = w_p.tile([HD, T], BF, tag="sins", name="sins")
                xt, wq_t, wk_t, wv_t = [], [], [], []
                def load_wq(kc):
                    a = w_p.tile([128, H * HD], BF, tag=f"wq{kc}", name=f"wq{kc}")
                    nc.sync.dma_start(out=a, in_=wq_d[kc * 128:(kc + 1) * 128, :])
                    wq_t.append(a)

                for kc in range(KC):
                    b_ = w_p.tile([128, HD], BF, tag=f"wk{kc}", name=f"wk{kc}")
                    nc.sync.dma_start(out=b_, in_=wk_d[kc * 128:(kc + 1) * 128, :])
                    wk_t.append(b_)
                    c = w_p.tile([128, HD], BF, tag=f"wv{kc}", name=f"wv{kc}")
                    nc.sync.dma_start(out=c, in_=wv_d[kc * 128:(kc + 1) * 128, :])
                    wv_t.append(c)
                    t_ = xt_p.tile([128, T], BF, tag=f"xt{kc}", name=f"xt{kc}")
                    if kc < 2:
                        # quarter-DMAs so the first matmuls start as soon as the
                        # first 512 columns land
                        for tf in range(4):
                            nc.sync.dma_start(
                                out=t_[:, tf * 512:(tf + 1) * 512],
                                in_=xt_d[kc * 128:(kc + 1) * 128, tf * 512:(tf + 1) * 512])
                    else:
                        nc.sync.dma_start(out=t_, in_=xt_d[kc * 128:(kc + 1) * 128, :])
                    xt.append(t_)
                    if kc == 3:
                        nc.sync.dma_start(out=gqs, in_=gqs_d[:, :])
                        nc.sync.dma_start(out=gks, in_=gks_d[:, :])
                    if kc == 7:
                        nc.sync.dma_start(out=cosf, in_=cos_d[:, :])
                        nc.sync.dma_start(out=sins, in_=sin_d[:, :])
                for kc in range(KC):
                    load_wq(kc)
                for h in range(H):
                    nc.sync.dma_start(out=wo_t[h], in_=wo_d[h * 128:(h + 1) * 128, :])
                nc.sync.dma_start(out=mask, in_=msk_d[:, :])

                def project_ps(lhsT_of):
                    """pss[tf] = sum_kc lhsT(kc).T @ xt[kc][:, tf*512:+512].
                    kc-outer with 4 live psums so consecutive matmuls share lhsT."""
                    pss = [ps_proj.tile([128, 512], F32, tag="ps_proj", name=f"pp{tf}")
                           for tf in range(4)]
                    for kc in range(KC):
                        for tf in range(4):
                            nc.tensor.matmul(pss[tf], lhsT_of(kc),
                                             xt[kc][:, tf * 512:(tf + 1) * 512],
                                             start=(kc == 0), stop=(kc == KC - 1))
                    return pss

                def norm_pre_tf(pss, gain_col, sq, gt, sw, t1, tf):
                    """One 512-chunk of square + gain + rotate-half + cos/sin
                    combine, reading the projection PSUM directly."""
                    c = slice(tf * 512, (tf + 1) * 512)
                    nc.scalar.square(out=sq[:, c], in_=pss[tf])
                    nc.scalar.mul(gt[:, c], pss[tf], gain_col)
                    nc.sync.dma_start(out=sw[0:64, c], in_=gt[64:128, c])
                    nc.sync.dma_start(out=sw[64:128, c], in_=gt[0:64, c])
                    nc.vector.tensor_mul(t1[:, c], gt[:, c], cosf[:, c])
                    nc.gpsimd.tensor_mul(sw[:, c], sw[:, c], sins[:, c])
                    nc.vector.tensor_add(t1[:, c], t1[:, c], sw[:, c])

                def norm_fin_tf(sq, t1, hat_out, tf):
                    """One 512-chunk of rowsum-broadcast + rsqrt + final scaling."""
                    c = slice(tf * 512, (tf + 1) * 512)
                    pr = ps_proj.tile([128, 512], F32, tag="pr", name="pr", bufs=2)
                    nc.tensor.matmul(pr, ones128, sq[:, c], start=True, stop=True)
                    rbv = tmp_p.tile([128, 512], F32, tag="rbv", name="rbv", bufs=3)
                    nc.scalar.activation(out=rbv, in_=pr, func=AF.Sqrt,
                                         bias=eps128[:, 0:1], scale=1.0 / HD)
                    nc.vector.reciprocal_approx_fast(out=rbv, in_=rbv)
                    nc.vector.tensor_mul(hat_out[:, c], t1[:, c], rbv)

                # K + V fully interleaved kc-outer (8 live psums: K x4 + V01 x2
                # on the proj ring, V23 x2 on the pr ring): 8 matmuls per chunk
                # keeps the PE saturated while xt streams in.
                psK = [ps_proj.tile([128, 512], F32, tag="ps_proj", name=f"pk{tf}")
                       for tf in range(4)]
                psV = [ps_proj.tile([128, 512], F32, tag="ps_proj", name=f"pv{tf}")
                       for tf in range(2)]
                psV2 = [ps_proj.tile([128, 512], F32, tag="pr", name=f"pv2{tf}", bufs=2)
                        for tf in range(2)]
                for kc in range(KC):
                    for tf in range(4):
                        nc.tensor.matmul(psK[tf], wk_t[kc],
                                         xt[kc][:, tf * 512:(tf + 1) * 512],
                                         start=(kc == 0), stop=(kc == KC - 1))
                    for tf in range(2):
                        nc.tensor.matmul(psV[tf], wv_t[kc],
                                         xt[kc][:, tf * 512:(tf + 1) * 512],
                                         start=(kc == 0), stop=(kc == KC - 1))
                    for tf in range(2):
                        nc.tensor.matmul(psV2[tf], wv_t[kc],
                                         xt[kc][:, (tf + 2) * 512:(tf + 3) * 512],
                                         start=(kc == 0), stop=(kc == KC - 1))
                sqK = tmp_p.tile([128, T], BF, tag="sq", name="sqK", bufs=2)
                gtK = tmp_p.tile([128, T], BF, tag="gt", name="gtK", bufs=2)
                swK = tmp_p.tile([128, T], BF, tag="sw", name="swK", bufs=2)
                t1K = tmp_p.tile([128, T], BF, tag="t1", name="t1K", bufs=2)
                for tf in range(4):
                    norm_pre_tf(psK, gks[:, 0:1], sqK, gtK, swK, t1K, tf)
                vtr = tmp_p.tile([128, T], BF, tag="vtr", name="vtr", bufs=1)
                for tf in range(2):
                    nc.scalar.copy(out=vtr[:, tf * 512:(tf + 1) * 512], in_=psV[tf])
                    nc.scalar.copy(out=vtr[:, (tf + 2) * 512:(tf + 3) * 512], in_=psV2[tf])
                # V -> natural layout via DMA xbar transposes (no PE/PSUM use)
                for tb in range(TB):
                    nc.sync.dma_start_transpose(
                        out=vnat[:, tb * 128:(tb + 1) * 128],
                        in_=vtr[:, tb * 128:(tb + 1) * 128])

                # Q heads: each head's norm_pre chunks interleave with the
                # PREVIOUS head's norm_fin chunks, so every sqrt sits right
                # behind its rowsum in the scalar queue (no HOL blocking).
                # Q3's norm_pre writes persistent tiles; its norm_fin is
                # emitted in phase 2 between ic0 head streams.
                prev = (sqK, t1K, khat)
                for h in range(H):
                    pss = project_ps(lambda kc: wq_t[kc][:, h * 128:(h + 1) * 128])
                    if h == H - 1:
                        sq, t1 = sq3, t13
                    else:
                        sq = tmp_p.tile([128, T], BF, tag="sq", name=f"sq{h}", bufs=2)
                        t1 = tmp_p.tile([128, T], BF, tag="t1", name=f"t1{h}", bufs=2)
                    gt = tmp_p.tile([128, T], BF, tag="gt", name=f"gt{h}", bufs=2)
                    sw = tmp_p.tile([128, T], BF, tag="sw", name=f"sw{h}", bufs=2)
                    for tf in range(4):
                        norm_pre_tf(pss, gqs[:, h:h + 1], sq, gt, sw, t1, tf)
                        norm_fin_tf(*prev, tf)
                    prev = (sq, t1, qhat[h])

            # ------- Phases 2+3: causal attention with pipelined output projection ---
            # ic ascending; the output projection of chunk ic-1 is emitted as filler
            # into chunk ic's QK/exp/PV stream so exp latency never idles the PE.
            with ExitStack() as s2:
                o_p = s2.enter_context(tc.tile_pool(name="op", bufs=1))
                oT = [o_p.tile([128, T], BF, tag=f"oT{h}", name=f"oT{h}") for h in range(H)]
                p_p = s2.enter_context(tc.tile_pool(name="pexp", bufs=72))
                db_p = s2.enter_context(tc.tile_pool(name="dbv", bufs=2))
                ost_p = s2.enter_context(tc.tile_pool(name="ost", bufs=6))
                ps_acc = s2.enter_context(tc.tile_pool(name="ps_acc", bufs=4, space="PSUM"))
                ps_s = s2.enter_context(tc.tile_pool(name="ps_s", bufs=3, space="PSUM"))
                ps_os = s2.enter_context(tc.tile_pool(name="ps_os", bufs=1, space="PSUM"))

                def fin_q3_tf(tf):
                    # Q3's rmsnorm finish, one 512-chunk at a time, interleaved
                    # between ic0 head streams (ic0 reads only qhat[3][:, 0:512])
                    c = slice(tf * 512, (tf + 1) * 512)
                    pr = ps_os.tile([128, 512], F32, tag="pso", name="pr3")
                    nc.tensor.matmul(pr, ones128, sq3[:, c], start=True, stop=True)
                    rbv = db_p.tile([128, 512], F32, tag="dbv", name="rbv3")
                    nc.scalar.activation(out=rbv, in_=pr, func=AF.Sqrt,
                                         bias=eps128[:, 0:1], scale=1.0 / HD)
                    nc.vector.reciprocal_approx_fast(out=rbv, in_=rbv)
                    nc.vector.tensor_mul(qhat[3][:, c], t13[:, c], rbv)

                def outproj_thunks(ic, tail=False):
                    """Emission thunks for chunk ic's output projection: per (tb, oc)
                    a 4-matmul h-chain into one PSUM bank, then copy+DMA. The tail
                    run alternates with the (then idle) scores ring to pipeline."""
                    thunks = []
                    idx = 0
                    for tb in range(4 * ic, 4 * ic + 4):
                        for oc in range(4):
                            use_sc = tail and (idx % 2 == 1)

                            def chain(tb=tb, oc=oc, use_sc=use_sc):
                                if use_sc:
                                    pso = ps_s.tile([128, 512], F32, tag="sc", name="pso")
                                else:
                                    pso = ps_os.tile([128, 512], F32, tag="pso", name="pso")
                                for h in range(H):
                                    nc.tensor.matmul(pso, oT[h][:, tb * 128:(tb + 1) * 128],
                                                     wo_t[h][:, oc * 512:(oc + 1) * 512],
                                                     start=(h == 0), stop=(h == H - 1))
                                ost = ost_p.tile([128, 512], BF, tag="ost", name="ost")
                                nc.vector.tensor_copy(out=ost, in_=pso)
                                nc.sync.dma_start(
                                    out=out_d[tb * 128:(tb + 1) * 128,
                                              oc * 512:(oc + 1) * 512], in_=ost)
                            thunks.append(chain)
                            idx += 1
                    return thunks

                filler = []
                for ic in range(IC):
                    i0 = ic * 512
                    jb_max = 4 * ic + 3
                    po = [ps_acc.tile([128, 512], F32, tag="acc", name=f"po{h}")
                          for h in range(H)]
                    offs = [max(0, 128 * (jb - 4 * ic)) for jb in range(jb_max + 1)]
                    # spread prev chunk's 16 outproj chains over this chunk's jbs
                    n_fill = len(filler)
                    popped = 0
                    pt = {}  # (h, jb) -> exp tile, consumed by PV now + db chain later

                    def qk_exp_pv(h, jb, start, stop):
                        off = offs[jb]
                        ps = ps_s.tile([128, 512], F32, tag="sc", name="sc")
                        nc.tensor.matmul(ps[:, off:], khat[:, jb * 128:(jb + 1) * 128],
                                         qhat[h][:, i0 + off:i0 + 512],
                                         start=True, stop=True)
                        if jb >= 4 * ic:
                            nc.vector.tensor_add(ps[:, off:off + 128],
                                                 ps[:, off:off + 128], mask)
                        p = p_p.tile([128, 512], BF, tag="p", name="p")
                        nc.scalar.activation(out=p[:, off:], in_=ps[:, off:], func=AF.Exp)
                        pt[(h, jb)] = p
                        nc.tensor.matmul(po[h][:, off:], vnat[:, jb * 128:(jb + 1) * 128],
                                         p[:, off:], start=start, stop=stop)

                    if ic == 0:
                        # h-major: early heads' QKs flow while the last Q heads'
                        # norm chains (phase-1 tail) are still finishing; Q3's
                        # deferred norm_fin chunks slot between the streams
                        for h in range(H):
                            for jb in range(jb_max + 1):
                                qk_exp_pv(h, jb, start=(jb == 0), stop=(jb == jb_max))
                            if h < 3:
                                fin_q3_tf(h)
                    else:
                        for jb in range(jb_max + 1):
                            off = offs[jb]
                            kb = khat[:, jb * 128:(jb + 1) * 128]
                            pss = []
                            for h in range(H):  # 4 QKs share the stationary khat block
                                ps = ps_s.tile([128, 512], F32, tag="sc", name="sc")
                                nc.tensor.matmul(ps[:, off:], kb,
                                                 qhat[h][:, i0 + off:i0 + 512],
                                                 start=True, stop=True)
                                if jb >= 4 * ic:
                                    nc.vector.tensor_add(ps[:, off:off + 128],
                                                         ps[:, off:off + 128], mask)
                                pss.append(ps)
                            for h in range(H):
                                p = p_p.tile([128, 512], BF, tag="p", name="p")
                                nc.scalar.activation(out=p[:, off:], in_=pss[h][:, off:],
                                                     func=AF.Exp)
                                pt[(h, jb)] = p
                            for h in range(H):  # 4 PVs share the stationary vnat block
                                nc.tensor.matmul(po[h][:, off:],
                                                 vnat[:, jb * 128:(jb + 1) * 128],
                                                 pt[(h, jb)][:, off:],
                                                 start=(jb == 0), stop=(jb == jb_max))
                            while popped < n_fill * (jb + 1) // (jb_max + 1):
                                filler.pop(0)()
                                popped += 1
                    for h in range(H):
                        # denominator chain with ones[128,128] lhsT: the PSUM result
                        # is the softmax denominator broadcast to all partitions.
                        dbp = ps_s.tile([128, 512], F32, tag="sc", name="dbp")
                        for jb in range(jb_max + 1):
                            nc.tensor.matmul(dbp[:, offs[jb]:], ones128,
                                             pt[(h, jb)][:, offs[jb]:],
                                             start=(jb == 0), stop=(jb == jb_max))
                        dbv = db_p.tile([128, 512], F32, tag="dbv", name="dbv")
                        nc.vector.reciprocal_approx_fast(out=dbv, in_=dbp)
                        nc.vector.tensor_mul(oT[h][:, i0:i0 + 512], po[h], dbv)
                    while filler:
                        filler.pop(0)()
                    if ic == 0:
                        fin_q3_tf(3)
                    filler = outproj_thunks(ic, tail=(ic == IC - 1))
                while filler:
                    filler.pop(0)()
    nc.finalize()
    return nc


def _rope_tables():
    d = np.arange(64, dtype=np.float64)
    ang = 10000.0 ** (-d / 64.0)
    pos = np.arange(T, dtype=np.float64)
    rad = pos[None, :] * ang[:, None]          # [64, T]
    cos, sin = np.cos(rad), np.sin(rad)
    cosF = np.concatenate([cos, cos], 0).astype(bf16)
    sinS = np.concatenate([-sin, sin], 0).astype(bf16)
    return np.ascontiguousarray(cosF), np.ascontiguousarray(sinS)


def _in_maps(x, wq, wk, wv, wo, gq, gk):
    cosF, sinS = _rope_tables()
    mask = np.ascontiguousarray(np.triu(np.full((128, 128), -1e9, np.float32), 1).T)
    maps = []
    for core in range(8):
        b, g = core // 4, core % 4
        maps.append({
            "xt": np.ascontiguousarray(x[b].T).astype(bf16),
            "wq": np.ascontiguousarray(wq[:, g * 512:(g + 1) * 512]).astype(bf16),
            "wk": np.ascontiguousarray(wk[:, g * 128:(g + 1) * 128]).astype(bf16),
            "wv": np.ascontiguousarray(wv[:, g * 128:(g + 1) * 128]).astype(bf16),
            "wo": np.ascontiguousarray(wo[g * 512:(g + 1) * 512, :]).astype(bf16),
            "gqs": np.ascontiguousarray((gq[g].T * MULT2).astype(np.float32)),
            "gks": np.ascontiguousarray(gk[g].astype(np.float32).reshape(HD, 1)),
            "cosf": cosF, "sins": sinS, "mask": mask,
        })
    return maps


def _get_nc():
    if "nc" not in _NC_CACHE:
        _NC_CACHE["nc"] = _build_nc()
    return _NC_CACHE["nc"]


def _run(inputs, trace=False, trace_kwargs=None, tmpdir=None):
    nc = _get_nc()
    maps = _in_maps(inputs["x"], inputs["wq"], inputs["wk"], inputs["wv"],
                    inputs["wo"], inputs["gq"], inputs["gk"])
    res = run_bass_kernel_spmd(nc, maps, core_ids=list(range(8)), trace=trace,
                               tmpdir=tmpdir, **(trace_kwargs or {}))
    out = np.zeros((B, T, D), np.float32)
    for core in range(8):
        out[core // 4] += res.results[core]["out"].astype(np.float32)
    return out, res


def kernel(**inputs):
    inputs = {k: np.asarray(v) for k, v in inputs.items()}
    out, _ = _run(inputs, trace=False)
    return out
